# revision 21
# baseline (speedup 1.0000x reference)
"""MoE layer (8 experts, top-2, SwiGLU) on 8 TRN2 NeuronCores.

Strategy: expert-parallel. The router (x @ Wr, top-2, softmax) runs on the
host — it is ~0.03% of the FLOPs. Tokens are dispatched per expert on the
host (the "all-to-all"), each core runs its expert's dense SwiGLU MLP over
its (capacity-padded) token batch, and the host applies the combine
weights (including DEPTH_SCALE) on the way back.

Device layout (per core / expert e):
  xt   [C, CAP]               gathered tokens, transposed (feature-major), fp16
  wg_t [H/128,128,C/128,128]  Wg[e] pre-tiled so each lhsT tile DMA is
  wu_t                        contiguous, fp16
  wd_t [C/128,128,H/128,128]  Wd[e] pre-tiled, fp16
  yt   [C, CAP]               (silu(x@Wg) * (x@Wu)) @ Wd, transposed,
                              unscaled, fp32

All matmuls are fp16 with fp32 PSUM accumulation (full PE rate, ~5e-4
end-to-end error — the harness gate is 2e-2). fp16 halves all weight/x
DMA and SBUF traffic vs fp32r at identical PE speed, which buys the SBUF
headroom for deep tile pools (the measured win over shallow pools) and
lets the hidden dimension stay whole (h is fp16, 68 KB/partition), so
there is a single gate/up phase, a single down phase, and one output.
"""

import os
import sys

if "/opt/trn_rl_repo" not in sys.path:
    sys.path.insert(0, "/opt/trn_rl_repo")

import numpy as np

# fp16 matmuls emit explicit InstLdweights, which walrus rejects under
# --enable-ldw-opt=true; the flag only helps self-loading fp32/fp32r
# matmuls, so it stays off for this kernel.
_LDW_OPT = os.environ.get("MOE_LDW_OPT", "0") == "1"
_ldw_patched = False


_POLICY = os.environ.get("MOE_POLICY", "")


def _enable_ldw_opt():
    """Patch walrus compile flags for our NEFF compiles.

    MOE_LDW_OPT=1 flips --enable-ldw-opt to true (only valid for
    self-loading fp32/fp32r matmuls — breaks fp16). MOE_POLICY=<n>
    replaces --policy=0 with the given post-scheduler policy
    (2: heuristics-based, 3: time-aware).
    """
    global _ldw_patched
    if _ldw_patched or (not _LDW_OPT and _POLICY == ""):
        return
    import concourse.bass_utils as bu

    orig = bu.run_command

    def patched(argv, **kw):
        if _LDW_OPT:
            argv = [
                "--enable-ldw-opt=true" if a == "--enable-ldw-opt=false"
                else a for a in argv
            ]
        if _POLICY:
            argv = [
                f"--policy={_POLICY}" if a == "--policy=0" else a
                for a in argv
            ]
        return orig(argv, **kw)

    patched.__wrapped_orig__ = orig
    bu.run_command = patched
    _ldw_patched = True


D_MODEL = 1024
HIDDEN = 4096
NUM_EXPERTS = 8
TOP_K = 2
DEPTH_SCALE = 1.0 / np.sqrt(12.0)

P = 128
NC = D_MODEL // P     # 8 column chunks of the model dim
NH = HIDDEN // P      # 32 chunks of the hidden dim


def _token_subtiles(cap):
    """Pad cap so it splits into equal even-width sub-tiles in [256, 512]
    (the PSUM bank caps fp32 matmul width at 512). Returns (padded_cap,
    subs)."""
    cap = max(256, cap)
    nt = max(1, -(-cap // 512))
    step = 2 * nt
    cap = -(-cap // step) * step
    size = cap // nt
    assert 256 <= size <= 512 and size % 2 == 0, (cap, size)
    return cap, [(i * size, size) for i in range(nt)]


def _build_nc(cap, repeat=1, wgu_bufs=4, wd_bufs=8, y_bufs=2, s_bufs=3,
              nt=None, x_bufs=1, interleave=0):
    import concourse.bass as bass
    import concourse.mybir as mybir
    import concourse.tile as tile
    from concourse import bacc

    F32 = mybir.dt.float32
    F16 = mybir.dt.float16
    if nt is None:
        cap2, subs = _token_subtiles(cap)
        assert cap2 == cap, (cap2, cap)
    else:
        # explicit subtile count: narrower tiles raise stationary reuse
        # (LDW amortization) at the cost of more PSUM tiles in flight
        assert cap % nt == 0, (cap, nt)
        size = cap // nt
        assert size <= 512
        subs = [(i * size, size) for i in range(nt)]
    nt = len(subs)
    ps_w = max(s for _, s in subs)

    nc = bacc.Bacc("TRN2", target_bir_lowering=False, debug=False,
                   num_devices=8)
    xt = nc.dram_tensor("xt", [D_MODEL, cap], F16, kind="ExternalInput").ap()
    wg_t = nc.dram_tensor("wg_t", [NH, P, NC, P], F16,
                          kind="ExternalInput").ap()
    wu_t = nc.dram_tensor("wu_t", [NH, P, NC, P], F16,
                          kind="ExternalInput").ap()
    wd_t = nc.dram_tensor("wd_t", [NC, P, NH, P], F16,
                          kind="ExternalInput").ap()
    yt = nc.dram_tensor("yt", [D_MODEL, cap], F32, kind="ExternalOutput").ap()

    from contextlib import ExitStack

    with tile.TileContext(nc) as tc:
        rep = ExitStack()
        if repeat > 1:
            rep.enter_context(tc.For_i(0, repeat, 1))
        with (
            rep,
            tc.tile_pool(name="xpool", bufs=x_bufs) as xpool,
            tc.tile_pool(name="hpool", bufs=1) as hpool,
            tc.tile_pool(name="wg", bufs=wgu_bufs) as wgp,
            tc.tile_pool(name="wu", bufs=wgu_bufs) as wup,
            tc.tile_pool(name="wd", bufs=wd_bufs) as wdp,
            tc.tile_pool(name="spool", bufs=s_bufs) as spool,
            tc.tile_pool(name="ypool", bufs=y_bufs) as ypool,
            tc.tile_pool(name="psum", bufs=8, space="PSUM") as psp,
        ):
            xt_sb = xpool.tile([P, NC, cap], F16)
            xt_r = xt.rearrange("(o p) n -> p o n", p=P)
            for c in range(NC):
                nc.sync.dma_start(xt_sb[:, c], xt_r[:, c])
            h_sb = hpool.tile([P, NH, cap], F16, tag="h")

            wd_all = []
            for hc in range(NH):
                wg_sb = wgp.tile([P, NC, P], F16, tag="wg")
                nc.sync.dma_start(wg_sb[:], wg_t[hc])
                wu_sb = wup.tile([P, NC, P], F16, tag="wu")
                nc.sync.dma_start(wu_sb[:], wu_t[hc])

                pg = [psp.tile([P, ps_w], F32, tag="ps", name=f"pg{hc}_{t}")
                      for t in range(nt)]
                pu = [psp.tile([P, ps_w], F32, tag="ps", name=f"pu{hc}_{t}")
                      for t in range(nt)]
                if interleave:
                    # 6 accumulation chains interleaved: dependent steps
                    # into the same PSUM bank are 6 instructions apart
                    for c in range(NC):
                        for ps_t in (pg, pu):
                            w_sb = wg_sb if ps_t is pg else wu_sb
                            for t, (o, w) in enumerate(subs):
                                nc.tensor.matmul(
                                    ps_t[t][:, :w],
                                    w_sb[:, c],
                                    xt_sb[:, c, o:o + w],
                                    start=(c == 0),
                                    stop=(c == NC - 1),
                                )
                else:
                    for c in range(NC):
                        for t, (o, w) in enumerate(subs):
                            nc.tensor.matmul(
                                pg[t][:, :w],
                                wg_sb[:, c],
                                xt_sb[:, c, o:o + w],
                                start=(c == 0),
                                stop=(c == NC - 1),
                            )
                    for c in range(NC):
                        for t, (o, w) in enumerate(subs):
                            nc.tensor.matmul(
                                pu[t][:, :w],
                                wu_sb[:, c],
                                xt_sb[:, c, o:o + w],
                                start=(c == 0),
                                stop=(c == NC - 1),
                            )
                for t, (o, w) in enumerate(subs):
                    s_sb = spool.tile([P, ps_w], F32, tag="s")
                    nc.scalar.activation(
                        s_sb[:, :w], pg[t][:, :w],
                        mybir.ActivationFunctionType.Silu,
                    )
                    nc.vector.tensor_mul(
                        h_sb[:, hc, o:o + w], s_sb[:, :w], pu[t][:, :w]
                    )
                if hc >= NH - NC:
                    # prefetch one down-weight tile per late gate iteration:
                    # each 1MB transfer slots between gate-weight DMAs
                    # (within prefetch slack), so all 8 are resident when
                    # the down phase starts
                    wd_sb = wdp.tile([P, NH, P], F16, tag="wd")
                    nc.sync.dma_start(wd_sb[:], wd_t[hc - (NH - NC)])
                    wd_all.append(wd_sb)

            oc_step = 2 if interleave else 1
            for oc0 in range(0, NC, oc_step):
                ocs = range(oc0, oc0 + oc_step)
                wd_sbs, pys = [], []
                for oc in ocs:
                    wd_sbs.append(wd_all[oc])
                    pys.append([
                        psp.tile([P, ps_w], F32, tag="ps", name=f"py{oc}_{t}")
                        for t in range(nt)
                    ])
                for hh in range(NH):
                    for wd_sb, py in zip(wd_sbs, pys):
                        for t, (o, w) in enumerate(subs):
                            nc.tensor.matmul(
                                py[t][:, :w],
                                wd_sb[:, hh],
                                h_sb[:, hh, o:o + w],
                                start=(hh == 0),
                                stop=(hh == NH - 1),
                            )
                for oc, py in zip(ocs, pys):
                    y_sb = ypool.tile([P, cap], F32, tag="y")
                    for t, (o, w) in enumerate(subs):
                        nc.vector.tensor_copy(y_sb[:, o:o + w], py[t][:, :w])
                    nc.sync.dma_start(yt[oc * P:(oc + 1) * P, :], y_sb[:])

    nc.compile()
    return nc


def _route(flat_x, Wr):
    """Host router: per-expert (token_idx, weight) with top-2 softmax."""
    n = flat_x.shape[0]
    logits = (flat_x @ Wr).astype(np.float32)
    ar = np.arange(n)
    i0 = logits.argmax(1)
    l0 = logits[ar, i0]
    masked = logits.copy()
    masked[ar, i0] = -np.inf
    i1 = masked.argmax(1)
    l1 = logits[ar, i1]
    e1 = np.exp((l1 - l0).astype(np.float32))
    w0 = 1.0 / (1.0 + e1)
    w1 = e1 / (1.0 + e1)
    experts = []
    for e in range(NUM_EXPERTS):
        m0 = i0 == e
        m1 = i1 == e
        idx = np.concatenate([ar[m0], ar[m1]])
        w = np.concatenate([w0[m0], w1[m1]]).astype(np.float32)
        experts.append((idx, w))
    return experts


def _prep_inputs(flat, Wg, Wu, Wd, experts, cap):
    """Per-core input dict: gathered fp16 tokens + pre-tiled fp16 weights."""
    C = flat.shape[1]
    in_maps = []
    for e in range(NUM_EXPERTS):
        idx, _ = experts[e]
        xt = np.zeros((C, cap), dtype=np.float16)
        xt[:, : len(idx)] = flat[idx].astype(np.float16).T
        wg_t = np.ascontiguousarray(
            Wg[e].astype(np.float16).reshape(NC, P, NH, P).transpose(2, 1, 0, 3)
        )
        wu_t = np.ascontiguousarray(
            Wu[e].astype(np.float16).reshape(NC, P, NH, P).transpose(2, 1, 0, 3)
        )
        wd_t = np.ascontiguousarray(
            Wd[e].astype(np.float16).reshape(NH, P, NC, P).transpose(2, 1, 0, 3)
        )
        in_maps.append({"xt": xt, "wg_t": wg_t, "wu_t": wu_t, "wd_t": wd_t})
    return in_maps


def kernel(x, Wr, Wg, Wu, Wd):
    from concourse.bass_utils import run_bass_kernel_spmd

    _enable_ldw_opt()

    B, T, C = x.shape
    x = np.asarray(x, dtype=np.float32)
    Wr = np.asarray(Wr, dtype=np.float32)
    Wg = np.asarray(Wg, dtype=np.float32)
    Wu = np.asarray(Wu, dtype=np.float32)
    Wd = np.asarray(Wd, dtype=np.float32)
    flat = x.reshape(-1, C)
    experts = _route(flat, Wr)

    n_max = max(len(idx) for idx, _ in experts)
    cap, _ = _token_subtiles(n_max)

    nc = _build_nc(cap)
    in_maps = _prep_inputs(flat, Wg, Wu, Wd, experts, cap)

    try:
        res = run_bass_kernel_spmd(nc, in_maps, core_ids=list(range(8)))
    except Exception:
        # failsafe: retry once with the stock compiler flags
        global _ldw_patched
        if _ldw_patched:
            import concourse.bass_utils as bu

            bu.run_command = bu.run_command.__wrapped_orig__
            _ldw_patched = False
            res = run_bass_kernel_spmd(nc, in_maps, core_ids=list(range(8)))
        else:
            raise

    out = np.zeros((B * T, C), dtype=np.float64)
    for e in range(NUM_EXPERTS):
        idx, w = experts[e]
        ye = res.results[e]["yt"].astype(np.float64).T[: len(idx)]
        out[idx] += (w.astype(np.float64) * DEPTH_SCALE)[:, None] * ye
    return out.astype(np.float32).reshape(B, T, C)


if __name__ == "__main__":
    import reference

    inputs = reference.setup_inputs()
    out = kernel(**{k: np.asarray(v) for k, v in inputs.items()})
    print("kernel output", out.shape, out.dtype)


# revision 24
# speedup vs baseline: 1.0235x; 1.0235x over previous
"""MoE layer (8 experts, top-2, SwiGLU) on 8 TRN2 NeuronCores.

Strategy: expert-parallel. The router (x @ Wr, top-2, softmax) runs on the
host — it is ~0.03% of the FLOPs. Tokens are dispatched per expert on the
host (the "all-to-all"), each core runs its expert's dense SwiGLU MLP over
its (capacity-padded) token batch, and the host applies the combine
weights (including DEPTH_SCALE) on the way back.

Device layout (per core / expert e):
  xt   [C, CAP]               gathered tokens, transposed (feature-major), fp16
  wg_t [H/128,128,C/128,128]  Wg[e] pre-tiled so each lhsT tile DMA is
  wu_t                        contiguous, fp16
  wd_t [C/128,128,H/128,128]  Wd[e] pre-tiled, fp16
  yt   [C, CAP]               (silu(x@Wg) * (x@Wu)) @ Wd, transposed,
                              unscaled, fp32

All matmuls are fp16 with fp32 PSUM accumulation (full PE rate, ~5e-4
end-to-end error — the harness gate is 2e-2). fp16 halves all weight/x
DMA and SBUF traffic vs fp32r at identical PE speed, which buys the SBUF
headroom for deep tile pools (the measured win over shallow pools) and
lets the hidden dimension stay whole (h is fp16, 68 KB/partition), so
there is a single gate/up phase, a single down phase, and one output.
"""

import os
import sys

if "/opt/trn_rl_repo" not in sys.path:
    sys.path.insert(0, "/opt/trn_rl_repo")

import numpy as np

# fp16 matmuls emit explicit InstLdweights, which walrus rejects under
# --enable-ldw-opt=true; the flag only helps self-loading fp32/fp32r
# matmuls, so it stays off for this kernel.
_LDW_OPT = os.environ.get("MOE_LDW_OPT", "0") == "1"
_ldw_patched = False


_POLICY = os.environ.get("MOE_POLICY", "")


def _enable_ldw_opt():
    """Patch walrus compile flags for our NEFF compiles.

    MOE_LDW_OPT=1 flips --enable-ldw-opt to true (only valid for
    self-loading fp32/fp32r matmuls — breaks fp16). MOE_POLICY=<n>
    replaces --policy=0 with the given post-scheduler policy
    (2: heuristics-based, 3: time-aware).
    """
    global _ldw_patched
    if _ldw_patched or (not _LDW_OPT and _POLICY == ""):
        return
    import concourse.bass_utils as bu

    orig = bu.run_command

    def patched(argv, **kw):
        if _LDW_OPT:
            argv = [
                "--enable-ldw-opt=true" if a == "--enable-ldw-opt=false"
                else a for a in argv
            ]
        if _POLICY:
            argv = [
                f"--policy={_POLICY}" if a == "--policy=0" else a
                for a in argv
            ]
        return orig(argv, **kw)

    patched.__wrapped_orig__ = orig
    bu.run_command = patched
    _ldw_patched = True


D_MODEL = 1024
HIDDEN = 4096
NUM_EXPERTS = 8
TOP_K = 2
DEPTH_SCALE = 1.0 / np.sqrt(12.0)

P = 128
NC = D_MODEL // P     # 8 column chunks of the model dim
NH = HIDDEN // P      # 32 chunks of the hidden dim


def _token_subtiles(cap):
    """Pad cap so it splits into equal even-width sub-tiles in [256, 512]
    (the PSUM bank caps fp32 matmul width at 512). Returns (padded_cap,
    subs)."""
    cap = max(256, cap)
    nt = max(1, -(-cap // 512))
    step = 2 * nt
    cap = -(-cap // step) * step
    size = cap // nt
    assert 256 <= size <= 512 and size % 2 == 0, (cap, size)
    return cap, [(i * size, size) for i in range(nt)]


def _build_nc(cap, repeat=1, wgu_bufs=4, wd_bufs=8, y_bufs=2, s_bufs=3,
              nt=None, x_bufs=1, interleave=0):
    import concourse.bass as bass
    import concourse.mybir as mybir
    import concourse.tile as tile
    from concourse import bacc

    F32 = mybir.dt.float32
    F16 = mybir.dt.float16
    if nt is None:
        cap2, subs = _token_subtiles(cap)
        assert cap2 == cap, (cap2, cap)
    else:
        # explicit subtile count: narrower tiles raise stationary reuse
        # (LDW amortization) at the cost of more PSUM tiles in flight
        assert cap % nt == 0, (cap, nt)
        size = cap // nt
        assert size <= 512
        subs = [(i * size, size) for i in range(nt)]
    nt = len(subs)
    ps_w = max(s for _, s in subs)

    nc = bacc.Bacc("TRN2", target_bir_lowering=False, debug=False,
                   num_devices=8)
    xt = nc.dram_tensor("xt", [D_MODEL, cap], F16, kind="ExternalInput").ap()
    wg_t = nc.dram_tensor("wg_t", [NH, P, NC, P], F16,
                          kind="ExternalInput").ap()
    wu_t = nc.dram_tensor("wu_t", [NH, P, NC, P], F16,
                          kind="ExternalInput").ap()
    wd_t = nc.dram_tensor("wd_t", [NC, P, NH, P], F16,
                          kind="ExternalInput").ap()
    yt = nc.dram_tensor("yt", [D_MODEL, cap], F32, kind="ExternalOutput").ap()

    from contextlib import ExitStack

    with tile.TileContext(nc) as tc:
        rep = ExitStack()
        if repeat > 1:
            rep.enter_context(tc.For_i(0, repeat, 1))
        with (
            rep,
            tc.tile_pool(name="xpool", bufs=x_bufs) as xpool,
            tc.tile_pool(name="hpool", bufs=1) as hpool,
            tc.tile_pool(name="wg", bufs=wgu_bufs) as wgp,
            tc.tile_pool(name="wu", bufs=wgu_bufs) as wup,
            tc.tile_pool(name="wd", bufs=wd_bufs) as wdp,
            tc.tile_pool(name="spool", bufs=s_bufs) as spool,
            tc.tile_pool(name="ypool", bufs=y_bufs) as ypool,
            tc.tile_pool(name="psum", bufs=8, space="PSUM") as psp,
        ):
            xt_sb = xpool.tile([P, NC, cap], F16)
            xt_r = xt.rearrange("(o p) n -> p o n", p=P)
            for c in range(NC):
                nc.sync.dma_start(xt_sb[:, c], xt_r[:, c])
            h_sb = hpool.tile([P, NH, cap], F16, tag="h")

            for hc in range(NH):
                wg_sb = wgp.tile([P, NC, P], F16, tag="wg")
                nc.sync.dma_start(wg_sb[:], wg_t[hc])
                wu_sb = wup.tile([P, NC, P], F16, tag="wu")
                nc.sync.dma_start(wu_sb[:], wu_t[hc])

                pg = [psp.tile([P, ps_w], F32, tag="ps", name=f"pg{hc}_{t}")
                      for t in range(nt)]
                pu = [psp.tile([P, ps_w], F32, tag="ps", name=f"pu{hc}_{t}")
                      for t in range(nt)]
                if interleave:
                    # 6 accumulation chains interleaved: dependent steps
                    # into the same PSUM bank are 6 instructions apart
                    for c in range(NC):
                        for ps_t in (pg, pu):
                            w_sb = wg_sb if ps_t is pg else wu_sb
                            for t, (o, w) in enumerate(subs):
                                nc.tensor.matmul(
                                    ps_t[t][:, :w],
                                    w_sb[:, c],
                                    xt_sb[:, c, o:o + w],
                                    start=(c == 0),
                                    stop=(c == NC - 1),
                                )
                else:
                    for c in range(NC):
                        for t, (o, w) in enumerate(subs):
                            nc.tensor.matmul(
                                pg[t][:, :w],
                                wg_sb[:, c],
                                xt_sb[:, c, o:o + w],
                                start=(c == 0),
                                stop=(c == NC - 1),
                            )
                    for c in range(NC):
                        for t, (o, w) in enumerate(subs):
                            nc.tensor.matmul(
                                pu[t][:, :w],
                                wu_sb[:, c],
                                xt_sb[:, c, o:o + w],
                                start=(c == 0),
                                stop=(c == NC - 1),
                            )
                for t, (o, w) in enumerate(subs):
                    s_sb = spool.tile([P, ps_w], F32, tag="s")
                    nc.scalar.activation(
                        s_sb[:, :w], pg[t][:, :w],
                        mybir.ActivationFunctionType.Silu,
                    )
                    nc.vector.tensor_mul(
                        h_sb[:, hc, o:o + w], s_sb[:, :w], pu[t][:, :w]
                    )

            oc_step = 2 if interleave else 1
            for oc0 in range(0, NC, oc_step):
                ocs = range(oc0, oc0 + oc_step)
                wd_sbs, pys = [], []
                for oc in ocs:
                    wd_sb = wdp.tile([P, NH, P], F16, tag="wd")
                    nc.sync.dma_start(wd_sb[:], wd_t[oc])
                    wd_sbs.append(wd_sb)
                    pys.append([
                        psp.tile([P, ps_w], F32, tag="ps", name=f"py{oc}_{t}")
                        for t in range(nt)
                    ])
                for hh in range(NH):
                    for wd_sb, py in zip(wd_sbs, pys):
                        for t, (o, w) in enumerate(subs):
                            nc.tensor.matmul(
                                py[t][:, :w],
                                wd_sb[:, hh],
                                h_sb[:, hh, o:o + w],
                                start=(hh == 0),
                                stop=(hh == NH - 1),
                            )
                for oc, py in zip(ocs, pys):
                    y_sb = ypool.tile([P, cap], F32, tag="y")
                    for t, (o, w) in enumerate(subs):
                        nc.vector.tensor_copy(y_sb[:, o:o + w], py[t][:, :w])
                    nc.sync.dma_start(yt[oc * P:(oc + 1) * P, :], y_sb[:])

    nc.compile()
    return nc


def _route(flat_x, Wr):
    """Host router: per-expert (token_idx, weight) with top-2 softmax."""
    n = flat_x.shape[0]
    logits = (flat_x @ Wr).astype(np.float32)
    ar = np.arange(n)
    i0 = logits.argmax(1)
    l0 = logits[ar, i0]
    masked = logits.copy()
    masked[ar, i0] = -np.inf
    i1 = masked.argmax(1)
    l1 = logits[ar, i1]
    e1 = np.exp((l1 - l0).astype(np.float32))
    w0 = 1.0 / (1.0 + e1)
    w1 = e1 / (1.0 + e1)
    experts = []
    for e in range(NUM_EXPERTS):
        m0 = i0 == e
        m1 = i1 == e
        idx = np.concatenate([ar[m0], ar[m1]])
        w = np.concatenate([w0[m0], w1[m1]]).astype(np.float32)
        experts.append((idx, w))
    return experts


def _prep_inputs(flat, Wg, Wu, Wd, experts, cap):
    """Per-core input dict: gathered fp16 tokens + pre-tiled fp16 weights."""
    C = flat.shape[1]
    in_maps = []
    for e in range(NUM_EXPERTS):
        idx, _ = experts[e]
        xt = np.zeros((C, cap), dtype=np.float16)
        xt[:, : len(idx)] = flat[idx].astype(np.float16).T
        wg_t = np.ascontiguousarray(
            Wg[e].astype(np.float16).reshape(NC, P, NH, P).transpose(2, 1, 0, 3)
        )
        wu_t = np.ascontiguousarray(
            Wu[e].astype(np.float16).reshape(NC, P, NH, P).transpose(2, 1, 0, 3)
        )
        wd_t = np.ascontiguousarray(
            Wd[e].astype(np.float16).reshape(NH, P, NC, P).transpose(2, 1, 0, 3)
        )
        in_maps.append({"xt": xt, "wg_t": wg_t, "wu_t": wu_t, "wd_t": wd_t})
    return in_maps


def kernel(x, Wr, Wg, Wu, Wd):
    from concourse.bass_utils import run_bass_kernel_spmd

    _enable_ldw_opt()

    B, T, C = x.shape
    x = np.asarray(x, dtype=np.float32)
    Wr = np.asarray(Wr, dtype=np.float32)
    Wg = np.asarray(Wg, dtype=np.float32)
    Wu = np.asarray(Wu, dtype=np.float32)
    Wd = np.asarray(Wd, dtype=np.float32)
    flat = x.reshape(-1, C)
    experts = _route(flat, Wr)

    n_max = max(len(idx) for idx, _ in experts)
    cap, _ = _token_subtiles(n_max)

    nc = _build_nc(cap)
    in_maps = _prep_inputs(flat, Wg, Wu, Wd, experts, cap)

    try:
        res = run_bass_kernel_spmd(nc, in_maps, core_ids=list(range(8)))
    except Exception:
        # failsafe: retry once with the stock compiler flags
        global _ldw_patched
        if _ldw_patched:
            import concourse.bass_utils as bu

            bu.run_command = bu.run_command.__wrapped_orig__
            _ldw_patched = False
            res = run_bass_kernel_spmd(nc, in_maps, core_ids=list(range(8)))
        else:
            raise

    out = np.zeros((B * T, C), dtype=np.float64)
    for e in range(NUM_EXPERTS):
        idx, w = experts[e]
        ye = res.results[e]["yt"].astype(np.float64).T[: len(idx)]
        out[idx] += (w.astype(np.float64) * DEPTH_SCALE)[:, None] * ye
    return out.astype(np.float32).reshape(B, T, C)


if __name__ == "__main__":
    import reference

    inputs = reference.setup_inputs()
    out = kernel(**{k: np.asarray(v) for k, v in inputs.items()})
    print("kernel output", out.shape, out.dtype)
